# revision 2
# baseline (speedup 1.0000x reference)
"""GQA attention with 2D RoPE on 8 TRN2 NeuronCores — v3.

Sharding: batch data-parallel x4  X  head-group tensor-parallel x2.
Core c handles batch b=c//2 and head group g=c%2 (16 Q heads, 4 KV heads).
wo is row-sharded; the cross-pair reduction is done with two ROW-wise
ReduceScatters that write straight into the output tensor (no readback):
  RS1 over q rows [0:512)  -> even core y[0:256) = rows 0:256,
                              odd  core y[0:256) = rows 256:512
  RS2 over q rows [512:896) -> even y[256:448) = rows 512:704,
                               odd  y[256:448) = rows 704:896

v3 vs v2:
  - attention processed in two query passes (q 0:512 / 512:896) with
    exact-width causal score chunks (512-wide groups): 1008 matmuls
    total vs 1264.
  - causal masking via affine_select directly on the 128-wide diagonal
    blocks of U (no mask tiles).
  - O-projection for pass-1 rows zipped into pass-2 attention so RS1
    fires mid-pass-2; RS2 fires right after the last O tile; the
    collectives write y directly (zero tail after RS2).
  - Q projection (qblk 2-7) and WOB loads zipped into pass 1.
"""

import math
import numpy as np

import concourse.bass as bass
import concourse.tile as tile
import concourse.mybir as mybir
from concourse import bacc
from concourse import bass_utils

F32 = mybir.dt.float32
BF16 = mybir.dt.bfloat16
AF = mybir.ActivationFunctionType
ALU = mybir.AluOpType

B, L, D = 4, 896, 2048
HQ, HKV, HD = 32, 8, 64
NCORES = 8
GO = D // 2          # 1024 q-out dims per core
KVO = HKV * HD // 2  # 256 kv-out dims per core
NH = 16              # q heads per core
NKV = 4              # kv heads per core
P = 128
NI = D // P          # 16 contraction chunks
LB = L // P          # 7 key blocks
TOKC = ((0, 512), (512, 384))   # token chunks

# attention passes: (q0, qw, score tiles); tile = (kbs, slot_width)
PASSES = (
    (0, 512, (((0, 1), 512), ((2, 3), 256))),
    (512, 384, (((0, 1), 384), ((2, 3), 384), ((4, 5), 384), ((6,), 128))),
)

_NC_CACHE = {}
DEBUG_SKIP_AV = set()    # pass-2 kbs to drop from AV (debug only)
DEBUG_SKIP_SCORE = set()  # pass-2 kbs to skip scoring (debug only)


def build_nc(with_collective=True):
    key = (with_collective, tuple(sorted(DEBUG_SKIP_AV)),
           tuple(sorted(DEBUG_SKIP_SCORE)))
    if key in _NC_CACHE:
        return _NC_CACHE[key]
    nc = bacc.Bacc("TRN2", target_bir_lowering=False, debug=False,
                   num_devices=NCORES)
    ins = {
        "xT": nc.dram_tensor("xT", [D, L], BF16, kind="ExternalInput").ap(),
        "wqT": nc.dram_tensor("wqT", [D, GO], BF16, kind="ExternalInput").ap(),
        "wkvT": nc.dram_tensor("wkvT", [D, 2 * KVO], BF16,
                               kind="ExternalInput").ap(),
        "woT": nc.dram_tensor("woT", [GO, D], BF16, kind="ExternalInput").ap(),
        "Ct": nc.dram_tensor("Ct", [P, L], BF16, kind="ExternalInput").ap(),
        "St": nc.dram_tensor("St", [P, L], BF16, kind="ExternalInput").ap(),
    }
    y = nc.dram_tensor("y", [448, D], BF16, kind="ExternalOutput").ap()
    with tile.TileContext(nc) as tc:
        _build_kernel(nc, tc, ins, y, with_collective)
    nc.compile()
    _NC_CACHE[key] = nc
    return nc


def _ap3(dram_ap, row0, nrow_groups, group, ncols):
    """[128, nrow_groups, ncols] AP over dram rows row0.. in groups of 128."""
    return bass.AP(tensor=dram_ap.tensor,
                   offset=dram_ap.offset + row0 * ncols,
                   ap=[[ncols, P], [group, nrow_groups], [1, ncols]])


def _rope(nc, pool, t, C, S):
    """t = t*C + shuffle16(t)*S, fully in place."""
    shuf = pool.tile([P, L], BF16, tag="rope_shuf", name="rope_shuf")
    mask = [(p ^ 16) for p in range(32)]
    nc.vector.stream_shuffle(shuf[:], t[:], mask)
    nc.vector.tensor_mul(t[:], t[:], C[:])
    nc.vector.tensor_mul(shuf[:], shuf[:], S[:])
    nc.vector.tensor_add(t[:], t[:], shuf[:])


def _build_kernel(nc, tc, ins, y, with_collective):
    import contextlib
    ctx = contextlib.ExitStack()
    with ctx:
        const = ctx.enter_context(tc.tile_pool(name="const", bufs=1))
        big = ctx.enter_context(tc.tile_pool(name="big", bufs=1))
        WOB = big.tile([P, 8, D], BF16, tag="wob", name="wob")
        QT = [big.tile([P, L], BF16, tag=f"qt{i}", name=f"qt{i}")
              for i in range(8)]
        KTd = [big.tile([P, L], BF16, tag=f"kt{i}", name=f"kt{i}")
               for i in range(NKV)]
        Vext = [[[big.tile([P, P], BF16, tag=f"v{k}_{b_}_{vr}",
                           name=f"v{k}_{b_}_{vr}")
                  for vr in range(2)] for b_ in range(LB)] for k in range(NKV)]
        AT = [big.tile([P, L], BF16, tag=f"at{i}", name=f"at{i}")
              for i in range(8)]

        ev = ctx.enter_context(tc.tile_pool(name="ev", bufs=2))
        upool = ctx.enter_context(tc.tile_pool(name="uatt", bufs=2))
        recpool = ctx.enter_context(tc.tile_pool(name="rec", bufs=2))
        otpool = ctx.enter_context(tc.tile_pool(name="ot", bufs=3))
        ccdram = ctx.enter_context(tc.tile_pool(name="ccdram", bufs=1,
                                                space="DRAM"))
        cc1 = ccdram.tile([512, D], BF16, tag="cc1", name="cc1")
        cc2 = ccdram.tile([384, D], BF16, tag="cc2", name="cc2")
        cc1o = ccdram.tile([256, D], BF16, tag="cc1o", name="cc1o")
        cc2o = ccdram.tile([192, D], BF16, tag="cc2o", name="cc2o")

        proj = tc.tile_pool(name="proj", bufs=1)
        prj = proj.__enter__()
        XT = prj.tile([P, NI, L], BF16, tag="xt", name="xt")
        WKV = prj.tile([P, NI, 2 * KVO], BF16, tag="wkv", name="wkv")
        WQ = prj.tile([P, NI, GO], BF16, tag="wq", name="wq")

        # ---- rope tables ----
        C = const.tile([P, L], BF16, tag="C", name="C")
        S = const.tile([P, L], BF16, tag="S", name="S")
        # ---- input DMAs (issue order = priority) ----
        nc.sync.dma_start(WKV[:, 0:1, :],
                          _ap3(ins["wkvT"], 0, 1, P * 2 * KVO, 2 * KVO))
        nc.sync.dma_start(XT[:, 0:1, :], _ap3(ins["xT"], 0, 1, P * L, L))
        nc.sync.dma_start(XT[:, 1:2, :], _ap3(ins["xT"], 128, 1, P * L, L))
        nc.sync.dma_start(C[:], ins["Ct"])
        nc.sync.dma_start(S[:], ins["St"])
        nc.sync.dma_start(WKV[:, 1:4, :],
                          _ap3(ins["wkvT"], 128, 3, P * 2 * KVO, 2 * KVO))
        for c8 in range(1, 8):
            nc.sync.dma_start(XT[:, 2 * c8:2 * c8 + 2, :],
                              _ap3(ins["xT"], 256 * c8, 2, P * L, L))
            if c8 < 4:
                nc.sync.dma_start(
                    WKV[:, 4 * c8:4 * c8 + 4, :],
                    _ap3(ins["wkvT"], 512 * c8, 4, P * 2 * KVO, 2 * KVO))
        for c2 in range(2):
            nc.sync.dma_start(WQ[:, 8 * c2:8 * c2 + 8, :],
                              _ap3(ins["wqT"], 1024 * c2, 8, P * GO, GO))
        # ones halves of Vext + causal triangle mask (Pool, idle at start)
        TRI = const.tile([P, P], BF16, tag="tri", name="tri")
        nc.gpsimd.memset(TRI[:], 1.0)
        nc.gpsimd.affine_select(out=TRI[:], in_=TRI[:],
                                compare_op=ALU.is_ge, fill=0.0, base=0,
                                channel_multiplier=-1, pattern=[[1, P]])
        for k in range(NKV):
            for b_ in range(LB):
                nc.gpsimd.memset(Vext[k][b_][0][:, 64:128], 1.0)
                nc.gpsimd.memset(Vext[k][b_][1][:, 0:64], 1.0)

        # ---------------- phase 1: V, K, Q(qblk 0-1) ---------------------
        with tc.tile_pool(name="ph1", bufs=1, space="PSUM") as ph1:
            psv = [ph1.tile([P, 512], F32, tag=f"p{j}", name=f"pv{j}")
                   for j in range(LB)]
            for i in range(NI):
                st, sp = (i == 0), (i == NI - 1)
                for b_ in range(LB):
                    nc.tensor.matmul(
                        psv[b_][:, 0:KVO], XT[:, i, b_ * P:(b_ + 1) * P],
                        WKV[:, i, KVO:2 * KVO], start=st, stop=sp)
            for b_ in (4, 5, 6, 0, 1, 2, 3):
                for k in range(NKV):
                    sl = psv[b_][:, k * 64:(k + 1) * 64]
                    nc.vector.tensor_copy(Vext[k][b_][0][:, 0:64], sl)
                    nc.scalar.copy(Vext[k][b_][1][:, 64:128], sl)

            psk = [ph1.tile([P, 512], F32, tag=f"p{4 + ob * 2 + tci}",
                            name=f"pk{ob}_{tci}")
                   for ob in range(2) for tci, (t0, tw) in enumerate(TOKC)]
            for i in range(NI):
                st, sp = (i == 0), (i == NI - 1)
                for ob in range(2):
                    for tci, (t0, tw) in enumerate(TOKC):
                        nc.tensor.matmul(
                            psk[ob * 2 + tci][:, 0:tw],
                            WKV[:, i, ob * P:(ob + 1) * P],
                            XT[:, i, t0:t0 + tw], start=st, stop=sp)
            for ob in range(2):
                roped = ev.tile([P, L], BF16, tag="roped", name=f"ktall{ob}")
                for tci, (t0, tw) in enumerate(TOKC):
                    nc.scalar.copy(roped[:, t0:t0 + tw],
                                   psk[ob * 2 + tci][:, 0:tw])
                _rope(nc, ev, roped, C, S)
                for sub in range(2):
                    k = ob * 2 + sub
                    src = roped[sub * 64:(sub + 1) * 64, :]
                    nc.sync.dma_start(KTd[k][0:64, :], src)
                    nc.sync.dma_start(KTd[k][64:128, :], src)

            psq01 = [ph1.tile([P, 512], F32, tag=f"p{qb * 2 + tci}",
                              name=f"pq{qb}_{tci}")
                     for qb in range(2) for tci, (t0, tw) in enumerate(TOKC)]
            for i in range(NI):
                st, sp = (i == 0), (i == NI - 1)
                for qb in range(2):
                    for tci, (t0, tw) in enumerate(TOKC):
                        nc.tensor.matmul(
                            psq01[qb * 2 + tci][:, 0:tw],
                            WQ[:, i, qb * P:(qb + 1) * P],
                            XT[:, i, t0:t0 + tw], start=st, stop=sp)
            for qb in range(2):
                for tci, (t0, tw) in enumerate(TOKC):
                    nc.scalar.copy(QT[qb][:, t0:t0 + tw],
                                   psq01[qb * 2 + tci][:, 0:tw])
                _rope(nc, ev, QT[qb], C, S)

        # ---------------- attention-phase psum pool ----------------------
        att_cm = tc.tile_pool(name="att", bufs=1, space="PSUM")
        ps = att_cm.__enter__()
        sctr = [0]

        def q_tile(qblk, tci):
            t0, tw = TOKC[tci]
            psq = qzp.tile([P, 512], F32, tag="qz", name=f"psq{qblk}_{tci}")
            for i in range(NI):
                nc.tensor.matmul(psq[:, 0:tw], WQ[:, i, qblk * P:(qblk + 1) * P],
                                 XT[:, i, t0:t0 + tw],
                                 start=(i == 0), stop=(i == NI - 1))
            nc.scalar.copy(QT[qblk][:, t0:t0 + tw], psq[:, 0:tw])
            if tci == 1:
                _rope(nc, ev, QT[qblk], C, S)

        def o_tile(tb, oc, cc, row0, cast):
            pso = pop.tile([P, 512], F32, tag=f"po{(tb * 4 + oc) % 2}",
                           name=f"pso{tb}_{oc}")
            for ic in range(8):
                nc.tensor.matmul(pso[:], AT[ic][:, tb * P:(tb + 1) * P],
                                 WOB[:, ic, oc * 512:(oc + 1) * 512],
                                 start=(ic == 0), stop=(ic == 7))
            ot = otpool.tile([P, 512], BF16, tag=f"ot{(tb * 4 + oc) % 3}",
                             name=f"ot{tb}_{oc}")
            if cast == 0:
                nc.vector.tensor_copy(ot[:], pso[:])
            else:
                nc.scalar.copy(ot[:], pso[:])
            nc.sync.dma_start(
                bass.AP(tensor=cc.tensor,
                        offset=cc.offset + (tb * P - row0) * D + oc * 512,
                        ap=[[D, P], [1, 512]]),
                ot[:])

        def att_head(h, q0, qw, tiles, zips=()):
            kv = h // 4
            qblk, qsub = divmod(h, 2)
            qoff, soff, vr = 64 * qsub, 64 - 64 * qsub, qsub
            zi = list(zips)
            Us = []
            for ti, (kbs, sw) in enumerate(tiles):
                ns = len(kbs)
                pss = ps.tile([P, ns, 512], F32, tag=f"sa{sctr[0] % 2}",
                              name=f"pss{h}_{q0}_{ti}")
                sctr[0] += 1
                for j, kb in enumerate(kbs):
                    w0 = max(q0, P * kb)
                    w = q0 + qw - w0
                    nc.tensor.matmul(
                        pss[:, j, 0:w],
                        KTd[kv][qoff:qoff + 64, kb * P:(kb + 1) * P],
                        QT[qblk][qoff:qoff + 64, w0:w0 + w],
                        start=True, stop=True, tile_position=(qoff, 0))
                if ti == 0 and zi:
                    zi.pop(0)()
                U = upool.tile([P, ns, sw], BF16, tag=f"u{ti % 2}",
                               name=f"u{h}_{q0}_{ti}")
                nc.scalar.activation(U[:, :, :], pss[:, :, 0:sw], AF.Exp,
                                     scale=0.125)
                for j, kb in enumerate(kbs):
                    if P * kb >= q0:   # diagonal block -> causal mask (DVE)
                        nc.vector.tensor_tensor(
                            U[:, j, 0:P], U[:, j, 0:P], TRI[:], op=ALU.mult)
                Us.append((kbs, U))
            for z in zi:
                z()
            nkb = sum(len(kbs) for kbs, _ in tiles)
            psav = ps.tile([P, 512], F32, tag=f"av{h % 2}",
                           name=f"psav{h}_{q0}")
            kbi = 0
            skip = DEBUG_SKIP_AV if q0 == 512 else set()
            nkb -= len(skip)
            for kbs, U in Us:
                for j, kb in enumerate(kbs):
                    if kb in skip:
                        continue
                    w0 = max(q0, P * kb)
                    w = q0 + qw - w0
                    off = w0 - q0
                    nc.tensor.matmul(
                        psav[:, off:off + w], Vext[kv][kb][vr][:],
                        U[:, j, 0:w], start=(kbi == 0), stop=(kbi == nkb - 1),
                        skip_group_check=True)
                    kbi += 1
            recs = recpool.tile([P, 512], F32, tag="recs",
                                name=f"recs{h}_{q0}")
            nc.vector.reciprocal(recs[soff:soff + 64, 0:qw],
                                 psav[soff:soff + 64, 0:qw])
            rec = recpool.tile([P, 512], F32, tag="rec", name=f"rec{h}_{q0}")
            nc.sync.dma_start(rec[qoff:qoff + 64, 0:qw],
                              recs[soff:soff + 64, 0:qw])
            nc.vector.tensor_tensor(
                AT[qblk][qoff:qoff + 64, q0:q0 + qw],
                psav[qoff:qoff + 64, 0:qw],
                rec[qoff:qoff + 64, 0:qw], op=ALU.mult)

        # ---------------- pass 1 + zipped Q proj / WOB loads -------------
        qz_cm = tc.tile_pool(name="qzp", bufs=1, space="PSUM")
        qzp = qz_cm.__enter__()
        q0, qw, tiles = PASSES[0]
        for h in range(NH):
            zips = []
            if h < 12:
                qblk, tci = 2 + h // 2, h % 2
                zips.append(lambda qb=qblk, t=tci: q_tile(qb, t))
            if 4 <= h < 8:
                c = h - 4
                zips.append(lambda cc_=c: nc.sync.dma_start(
                    WOB[:, 2 * cc_:2 * cc_ + 2, :],
                    _ap3(ins["woT"], 256 * cc_, 2, P * D, D)))
            att_head(h, q0, qw, tiles, zips)
        proj.__exit__(None, None, None)
        qz_cm.__exit__(None, None, None)
        po_cm = tc.tile_pool(name="pop", bufs=1, space="PSUM")
        pop = po_cm.__enter__()

        # ---------------- pass 2 + zipped pass-1 O proj ------------------
        q0, qw, tiles = PASSES[1]
        nz = 0
        for h in range(NH):
            zips = []
            nzh = 2 if h < 4 else (1 if h < 12 else 0)
            for _ in range(nzh):
                tb, oc = divmod(nz, 4)
                zips.append(lambda tb_=tb, oc_=oc, n=nz: o_tile(
                    tb_, oc_, cc1, 0, n % 2))
                nz += 1
            att_head(h, q0, qw, tiles, zips)
            if h == 11 and with_collective:
                nc.gpsimd.collective_compute(
                    "ReduceScatter", ALU.add,
                    replica_groups=[[0, 1], [2, 3], [4, 5], [6, 7]],
                    ins=[cc1.opt()], outs=[cc1o.opt()])

        # ---------------- pass-2 O proj + RS2 ----------------------------
        for tb in range(4, 7):
            for oc in range(4):
                o_tile(tb, oc, cc2, 512, (tb * 4 + oc) % 2)
        if with_collective:
            nc.gpsimd.collective_compute(
                "ReduceScatter", ALU.add,
                replica_groups=[[0, 1], [2, 3], [4, 5], [6, 7]],
                ins=[cc2.opt()], outs=[cc2o.opt()])
            nc.gpsimd.dma_start(
                bass.AP(tensor=y.tensor, offset=y.offset,
                        ap=[[D, 256], [1, D]]),
                cc1o.opt())
            nc.gpsimd.dma_start(
                bass.AP(tensor=y.tensor, offset=y.offset + 256 * D,
                        ap=[[D, 192], [1, D]]),
                cc2o.opt())
            po_cm.__exit__(None, None, None)
        else:
            # debug path: no reduction; copy this core's partials
            nc.sync.dma_start(
                bass.AP(tensor=y.tensor, offset=y.offset,
                        ap=[[D, 256], [1, D]]),
                bass.AP(tensor=cc1.tensor, offset=cc1.offset,
                        ap=[[D, 256], [1, D]]))
            nc.sync.dma_start(
                bass.AP(tensor=y.tensor, offset=y.offset + 256 * D,
                        ap=[[D, 192], [1, D]]),
                bass.AP(tensor=cc2.tensor, offset=cc2.offset,
                        ap=[[D, 192], [1, D]]))
            po_cm.__exit__(None, None, None)
        att_cm.__exit__(None, None, None)


# ---------------------------------------------------------------- host side
_ROPE_PERM = np.concatenate([
    np.arange(0, 32, 2), np.arange(1, 32, 2),
    np.arange(32, 64, 2), np.arange(33, 64, 2)])


def make_in_maps(x, wq, wk, wv, wo, temporal_pos, structural_pos):
    import ml_dtypes
    bf16 = ml_dtypes.bfloat16
    x = np.asarray(x, dtype=np.float32)
    wq = np.asarray(wq, dtype=np.float32)
    wk = np.asarray(wk, dtype=np.float32)
    wv = np.asarray(wv, dtype=np.float32)
    wo = np.asarray(wo, dtype=np.float32)
    pt = np.asarray(temporal_pos).astype(np.float64)
    ps = np.asarray(structural_pos).astype(np.float64)
    inv = 1.0 / (10000.0 ** (np.arange(16) / 16.0))
    ct, st = np.cos(pt[:, None] * inv).T, np.sin(pt[:, None] * inv).T
    cs, ss = np.cos(ps[:, None] * inv).T, np.sin(ps[:, None] * inv).T
    Ct = np.concatenate([ct, ct, cs, cs] * 2).astype(bf16)     # [128, 896]
    St = np.concatenate([-st, st, -ss, ss] * 2).astype(bf16)

    wq_p = wq.reshape(HQ, HD, D)[:, _ROPE_PERM, :].reshape(D, D)
    wk_p = wk.reshape(HKV, HD, D)[:, _ROPE_PERM, :].reshape(HKV * HD, D)
    wqT = np.ascontiguousarray(wq_p.T).astype(bf16)   # [D, D]
    wkT = np.ascontiguousarray(wk_p.T).astype(bf16)   # [D, 512]
    wvT = np.ascontiguousarray(wv.T).astype(bf16)     # [D, 512]
    woT = np.ascontiguousarray(wo.T).astype(bf16)     # [D, D]

    in_maps = []
    for c in range(NCORES):
        b, g = divmod(c, 2)
        wkv = np.concatenate([wkT[:, g * KVO:(g + 1) * KVO],
                              wvT[:, g * KVO:(g + 1) * KVO]], axis=1)
        in_maps.append({
            "xT": np.ascontiguousarray(x[b].T).astype(bf16),
            "wqT": np.ascontiguousarray(wqT[:, g * GO:(g + 1) * GO]),
            "wkvT": np.ascontiguousarray(wkv),
            "woT": np.ascontiguousarray(woT[g * GO:(g + 1) * GO, :]),
            "Ct": Ct,
            "St": St,
        })
    return in_maps


def kernel(x, wq, wk, wv, wo, temporal_pos, structural_pos, _trace=False):
    nc = build_nc(with_collective=True)
    in_maps = make_in_maps(x, wq, wk, wv, wo, temporal_pos, structural_pos)
    res = bass_utils.run_bass_kernel_spmd(
        nc, in_maps, core_ids=list(range(NCORES)), trace=_trace)
    out = np.empty((B, L, D), np.float32)
    for b in range(B):
        e = np.asarray(res.results[2 * b]["y"]).astype(np.float32)
        o = np.asarray(res.results[2 * b + 1]["y"]).astype(np.float32)
        out[b, 0:256] = e[0:256]
        out[b, 256:512] = o[0:256]
        out[b, 512:704] = e[256:448]
        out[b, 704:896] = o[256:448]
    kernel.last_result = res
    return out


# revision 4
# speedup vs baseline: 1.0215x; 1.0215x over previous
"""GQA attention with 2D RoPE on 8 TRN2 NeuronCores — v3.

Sharding: batch data-parallel x4  X  head-group tensor-parallel x2.
Core c handles batch b=c//2 and head group g=c%2 (16 Q heads, 4 KV heads).
wo is row-sharded; the cross-pair reduction is done with two ROW-wise
ReduceScatters that write straight into the output tensor (no readback):
  RS1 over q rows [0:512)  -> even core y[0:256) = rows 0:256,
                              odd  core y[0:256) = rows 256:512
  RS2 over q rows [512:896) -> even y[256:448) = rows 512:704,
                               odd  y[256:448) = rows 704:896

v3 vs v2:
  - attention processed in two query passes (q 0:512 / 512:896) with
    exact-width causal score chunks (512-wide groups): 1008 matmuls
    total vs 1264.
  - causal masking via affine_select directly on the 128-wide diagonal
    blocks of U (no mask tiles).
  - O-projection for pass-1 rows zipped into pass-2 attention so RS1
    fires mid-pass-2; RS2 fires right after the last O tile; the
    collectives write y directly (zero tail after RS2).
  - Q projection (qblk 2-7) and WOB loads zipped into pass 1.
"""

import math
import numpy as np

import concourse.bass as bass
import concourse.tile as tile
import concourse.mybir as mybir
from concourse import bacc
from concourse import bass_utils

F32 = mybir.dt.float32
BF16 = mybir.dt.bfloat16
AF = mybir.ActivationFunctionType
ALU = mybir.AluOpType

B, L, D = 4, 896, 2048
HQ, HKV, HD = 32, 8, 64
NCORES = 8
GO = D // 2          # 1024 q-out dims per core
KVO = HKV * HD // 2  # 256 kv-out dims per core
NH = 16              # q heads per core
NKV = 4              # kv heads per core
P = 128
NI = D // P          # 16 contraction chunks
LB = L // P          # 7 key blocks
TOKC = ((0, 512), (512, 384))   # token chunks

# attention passes: (q0, qw, score tiles); tile = (kbs, slot_width)
PASSES = (
    (0, 512, (((0, 1), 512), ((2, 3), 256))),
    (512, 384, (((0, 1), 384), ((2, 3), 384), ((4, 5), 384), ((6,), 128))),
)

_NC_CACHE = {}
DEBUG_SKIP_AV = set()    # pass-2 kbs to drop from AV (debug only)
DEBUG_SKIP_SCORE = set()  # pass-2 kbs to skip scoring (debug only)


def build_nc(with_collective=True):
    key = (with_collective, tuple(sorted(DEBUG_SKIP_AV)),
           tuple(sorted(DEBUG_SKIP_SCORE)))
    if key in _NC_CACHE:
        return _NC_CACHE[key]
    nc = bacc.Bacc("TRN2", target_bir_lowering=False, debug=False,
                   num_devices=NCORES)
    ins = {
        "xT": nc.dram_tensor("xT", [D, L], BF16, kind="ExternalInput").ap(),
        "wqT": nc.dram_tensor("wqT", [D, GO], BF16, kind="ExternalInput").ap(),
        "wkvT": nc.dram_tensor("wkvT", [D, 2 * KVO], BF16,
                               kind="ExternalInput").ap(),
        "woT": nc.dram_tensor("woT", [GO, D], BF16, kind="ExternalInput").ap(),
        "Ct": nc.dram_tensor("Ct", [P, L], BF16, kind="ExternalInput").ap(),
        "St": nc.dram_tensor("St", [P, L], BF16, kind="ExternalInput").ap(),
    }
    y = nc.dram_tensor("y", [448, D], BF16, kind="ExternalOutput").ap()
    with tile.TileContext(nc) as tc:
        _build_kernel(nc, tc, ins, y, with_collective)
    nc.compile()
    _NC_CACHE[key] = nc
    return nc


def _ap3(dram_ap, row0, nrow_groups, group, ncols):
    """[128, nrow_groups, ncols] AP over dram rows row0.. in groups of 128."""
    return bass.AP(tensor=dram_ap.tensor,
                   offset=dram_ap.offset + row0 * ncols,
                   ap=[[ncols, P], [group, nrow_groups], [1, ncols]])


def _rope(nc, pool, t, C, S):
    """t = t*C + shuffle16(t)*S, fully in place."""
    shuf = pool.tile([P, L], BF16, tag="rope_shuf", name="rope_shuf")
    mask = [(p ^ 16) for p in range(32)]
    nc.vector.stream_shuffle(shuf[:], t[:], mask)
    nc.vector.tensor_mul(t[:], t[:], C[:])
    nc.vector.tensor_mul(shuf[:], shuf[:], S[:])
    nc.vector.tensor_add(t[:], t[:], shuf[:])


def _build_kernel(nc, tc, ins, y, with_collective):
    import contextlib
    ctx = contextlib.ExitStack()
    with ctx:
        const = ctx.enter_context(tc.tile_pool(name="const", bufs=1))
        big = ctx.enter_context(tc.tile_pool(name="big", bufs=1))
        WOB = big.tile([P, 8, D], BF16, tag="wob", name="wob")
        QT = [big.tile([P, L], BF16, tag=f"qt{i}", name=f"qt{i}")
              for i in range(8)]
        KTd = [big.tile([P, L], BF16, tag=f"kt{i}", name=f"kt{i}")
               for i in range(NKV)]
        Vext = [[big.tile([P, 192], BF16, tag=f"v{k}_{b_}",
                          name=f"v{k}_{b_}")
                  for b_ in range(LB)] for k in range(NKV)]
        AT = [big.tile([P, L], BF16, tag=f"at{i}", name=f"at{i}")
              for i in range(8)]

        ev = ctx.enter_context(tc.tile_pool(name="ev", bufs=2))
        upool = ctx.enter_context(tc.tile_pool(name="uatt", bufs=3))
        recpool = ctx.enter_context(tc.tile_pool(name="rec", bufs=2))
        otpool = ctx.enter_context(tc.tile_pool(name="ot", bufs=3))
        ccdram = ctx.enter_context(tc.tile_pool(name="ccdram", bufs=1,
                                                space="DRAM"))
        cc1 = ccdram.tile([512, D], BF16, tag="cc1", name="cc1")
        cc2 = ccdram.tile([384, D], BF16, tag="cc2", name="cc2")
        cc1o = ccdram.tile([256, D], BF16, tag="cc1o", name="cc1o")
        cc2o = ccdram.tile([192, D], BF16, tag="cc2o", name="cc2o")

        proj = tc.tile_pool(name="proj", bufs=1)
        prj = proj.__enter__()
        XT = prj.tile([P, NI, L], BF16, tag="xt", name="xt")
        WKV = prj.tile([P, NI, 2 * KVO], BF16, tag="wkv", name="wkv")
        WQ = prj.tile([P, NI, GO], BF16, tag="wq", name="wq")

        # ---- rope tables ----
        C = const.tile([P, L], BF16, tag="C", name="C")
        S = const.tile([P, L], BF16, tag="S", name="S")
        # ---- input DMAs (issue order = priority) ----
        nc.sync.dma_start(WKV[:, 0:1, :],
                          _ap3(ins["wkvT"], 0, 1, P * 2 * KVO, 2 * KVO))
        nc.sync.dma_start(XT[:, 0:1, :], _ap3(ins["xT"], 0, 1, P * L, L))
        nc.sync.dma_start(XT[:, 1:2, :], _ap3(ins["xT"], 128, 1, P * L, L))
        nc.sync.dma_start(C[:], ins["Ct"])
        nc.sync.dma_start(S[:], ins["St"])
        nc.sync.dma_start(WKV[:, 1:4, :],
                          _ap3(ins["wkvT"], 128, 3, P * 2 * KVO, 2 * KVO))
        for c8 in range(1, 8):
            nc.sync.dma_start(XT[:, 2 * c8:2 * c8 + 2, :],
                              _ap3(ins["xT"], 256 * c8, 2, P * L, L))
            if c8 < 4:
                nc.sync.dma_start(
                    WKV[:, 4 * c8:4 * c8 + 4, :],
                    _ap3(ins["wkvT"], 512 * c8, 4, P * 2 * KVO, 2 * KVO))
        for c2 in range(2):
            nc.sync.dma_start(WQ[:, 8 * c2:8 * c2 + 8, :],
                              _ap3(ins["wqT"], 1024 * c2, 8, P * GO, GO))
        # act-table prewarm + ones halves of Vext + causal triangle mask
        warm = const.tile([P, 1], BF16, tag="warm", name="warm")
        nc.scalar.activation(warm[:], warm[:], AF.Exp, scale=0.125)
        TRI = const.tile([P, P], BF16, tag="tri", name="tri")
        nc.gpsimd.memset(TRI[:], 1.0)
        nc.gpsimd.affine_select(out=TRI[:], in_=TRI[:],
                                compare_op=ALU.is_ge, fill=0.0, base=0,
                                channel_multiplier=-1, pattern=[[1, P]])
        for k in range(NKV):
            for b_ in range(LB):
                nc.gpsimd.memset(Vext[k][b_][:, 0:64], 1.0)
                nc.gpsimd.memset(Vext[k][b_][:, 128:192], 1.0)

        # ---------------- phase 1: V, K, Q(qblk 0-1) ---------------------
        with tc.tile_pool(name="ph1", bufs=1, space="PSUM") as ph1:
            psv = [ph1.tile([P, 512], F32, tag=f"p{j}", name=f"pv{j}")
                   for j in range(LB)]
            for i in range(NI):
                st, sp = (i == 0), (i == NI - 1)
                for b_ in range(LB):
                    nc.tensor.matmul(
                        psv[b_][:, 0:KVO], XT[:, i, b_ * P:(b_ + 1) * P],
                        WKV[:, i, KVO:2 * KVO], start=st, stop=sp)
            for b_ in (4, 5, 6, 0, 1, 2, 3):
                for k in range(NKV):
                    sl = psv[b_][:, k * 64:(k + 1) * 64]
                    if k % 2 == 0:
                        nc.vector.tensor_copy(Vext[k][b_][:, 64:128], sl)
                    else:
                        nc.scalar.copy(Vext[k][b_][:, 64:128], sl)

            _ktag = (7, 4, 5, 6)
            psk = [ph1.tile([P, 512], F32, tag=f"p{_ktag[ob * 2 + tci]}",
                            name=f"pk{ob}_{tci}")
                   for ob in range(2) for tci, (t0, tw) in enumerate(TOKC)]
            for i in range(NI):
                st, sp = (i == 0), (i == NI - 1)
                for ob in range(2):
                    for tci, (t0, tw) in enumerate(TOKC):
                        nc.tensor.matmul(
                            psk[ob * 2 + tci][:, 0:tw],
                            WKV[:, i, ob * P:(ob + 1) * P],
                            XT[:, i, t0:t0 + tw], start=st, stop=sp)
            for ob in range(2):
                roped = ev.tile([P, L], BF16, tag="roped", name=f"ktall{ob}")
                for tci, (t0, tw) in enumerate(TOKC):
                    nc.scalar.copy(roped[:, t0:t0 + tw],
                                   psk[ob * 2 + tci][:, 0:tw])
                _rope(nc, ev, roped, C, S)
                for sub in range(2):
                    k = ob * 2 + sub
                    src = roped[sub * 64:(sub + 1) * 64, :]
                    nc.sync.dma_start(KTd[k][0:64, :], src)
                    nc.sync.dma_start(KTd[k][64:128, :], src)

            psq01 = [ph1.tile([P, 512], F32, tag=f"p{qb * 2 + tci}",
                              name=f"pq{qb}_{tci}")
                     for qb in range(2) for tci, (t0, tw) in enumerate(TOKC)]
            for i in range(NI):
                st, sp = (i == 0), (i == NI - 1)
                for qb in range(2):
                    for tci, (t0, tw) in enumerate(TOKC):
                        nc.tensor.matmul(
                            psq01[qb * 2 + tci][:, 0:tw],
                            WQ[:, i, qb * P:(qb + 1) * P],
                            XT[:, i, t0:t0 + tw], start=st, stop=sp)
            for qb in range(2):
                for tci, (t0, tw) in enumerate(TOKC):
                    nc.scalar.copy(QT[qb][:, t0:t0 + tw],
                                   psq01[qb * 2 + tci][:, 0:tw])
                _rope(nc, ev, QT[qb], C, S)

        # ---------------- attention-phase psum pool ----------------------
        att_cm = tc.tile_pool(name="att", bufs=1, space="PSUM")
        ps = att_cm.__enter__()
        sctr = [0]

        def q_tile(qblk, tci):
            t0, tw = TOKC[tci]
            psq = qzp.tile([P, 512], F32, tag="qz", name=f"psq{qblk}_{tci}")
            for i in range(NI):
                nc.tensor.matmul(psq[:, 0:tw], WQ[:, i, qblk * P:(qblk + 1) * P],
                                 XT[:, i, t0:t0 + tw],
                                 start=(i == 0), stop=(i == NI - 1))
            nc.scalar.copy(QT[qblk][:, t0:t0 + tw], psq[:, 0:tw])
            if tci == 1:
                _rope(nc, ev, QT[qblk], C, S)

        def o_tile(tb, oc, cc, row0, cast):
            pso = pop.tile([P, 512], F32, tag=f"po{(tb * 4 + oc) % 2}",
                           name=f"pso{tb}_{oc}")
            for ic in range(8):
                nc.tensor.matmul(pso[:], AT[ic][:, tb * P:(tb + 1) * P],
                                 WOB[:, ic, oc * 512:(oc + 1) * 512],
                                 start=(ic == 0), stop=(ic == 7))
            ot = otpool.tile([P, 512], BF16, tag=f"ot{(tb * 4 + oc) % 3}",
                             name=f"ot{tb}_{oc}")
            if cast == 0:
                nc.vector.tensor_copy(ot[:], pso[:])
            else:
                nc.scalar.copy(ot[:], pso[:])
            dq = nc.sync if (cast == 1 or cc is cc1) else nc.scalar
            dq.dma_start(
                bass.AP(tensor=cc.tensor,
                        offset=cc.offset + (tb * P - row0) * D + oc * 512,
                        ap=[[D, P], [1, 512]]),
                ot[:])

        def emit_scores(h, q0, qw, tiles, zips=()):
            kv = h // 4
            qblk, qsub = divmod(h, 2)
            qoff = 64 * qsub
            zi = list(zips)
            Us = []
            for ti, (kbs, sw) in enumerate(tiles):
                ns = len(kbs)
                pss = ps.tile([P, ns, 512], F32, tag=f"sa{sctr[0] % 2}",
                              name=f"pss{h}_{q0}_{ti}")
                sctr[0] += 1
                for j, kb in enumerate(kbs):
                    w0 = max(q0, P * kb)
                    w = q0 + qw - w0
                    nc.tensor.matmul(
                        pss[:, j, 0:w],
                        KTd[kv][qoff:qoff + 64, kb * P:(kb + 1) * P],
                        QT[qblk][qoff:qoff + 64, w0:w0 + w],
                        start=True, stop=True, tile_position=(qoff, 0))
                if ti == 0 and zi:
                    zi.pop(0)()
                U = upool.tile([P, ns, sw], BF16, tag=f"u{ti % 2}",
                               name=f"u{h}_{q0}_{ti}")
                nc.scalar.activation(U[:, :, :], pss[:, :, 0:sw], AF.Exp,
                                     scale=0.125)
                for j, kb in enumerate(kbs):
                    if P * kb >= q0:   # diagonal block -> causal mask (DVE)
                        nc.vector.tensor_tensor(
                            U[:, j, 0:P], U[:, j, 0:P], TRI[:], op=ALU.mult)
                Us.append((kbs, U))
            for z in zi:
                z()
            return (h, q0, qw, tiles, Us)

        def emit_av(state):
            h, q0, qw, tiles, Us = state
            kv = h // 4
            qblk, qsub = divmod(h, 2)
            qoff, soff, vr = 64 * qsub, 64 - 64 * qsub, qsub
            nkb = sum(len(kbs) for kbs, _ in tiles)
            psav = ps.tile([P, 512], F32, tag=f"av{h % 2}",
                           name=f"psav{h}_{q0}")
            kbi = 0
            for kbs, U in Us:
                for j, kb in enumerate(kbs):
                    w0 = max(q0, P * kb)
                    w = q0 + qw - w0
                    off = w0 - q0
                    nc.tensor.matmul(
                        psav[:, off:off + w],
                        Vext[kv][kb][:, 64 * (1 - vr):64 * (1 - vr) + 128],
                        U[:, j, 0:w], start=(kbi == 0), stop=(kbi == nkb - 1),
                        skip_group_check=True)
                    kbi += 1
            recs = recpool.tile([P, 512], F32, tag="recs",
                                name=f"recs{h}_{q0}")
            nc.vector.reciprocal(recs[soff:soff + 64, 0:qw],
                                 psav[soff:soff + 64, 0:qw])
            rec = recpool.tile([P, 512], F32, tag="rec", name=f"rec{h}_{q0}")
            nc.sync.dma_start(rec[qoff:qoff + 64, 0:qw],
                              recs[soff:soff + 64, 0:qw])
            nc.vector.tensor_tensor(
                AT[qblk][qoff:qoff + 64, q0:q0 + qw],
                psav[qoff:qoff + 64, 0:qw],
                rec[qoff:qoff + 64, 0:qw], op=ALU.mult)

        # ------- both passes, scores lead AV by one head -----------------
        qz_cm = tc.tile_pool(name="qzp", bufs=1, space="PSUM")
        qzp = qz_cm.__enter__()
        po_cm = None
        pend = None
        nz = 0
        for p_, (q0, qw, tiles) in enumerate(PASSES):
            for h in range(NH):
                zips = []
                if p_ == 0:
                    if 2 <= h < 14:
                        zips.append(lambda qb=2 + (h - 2) // 2,
                                    t=(h - 2) % 2: q_tile(qb, t))
                    if 4 <= h < 8:
                        c = h - 4
                        zips.append(lambda cc_=c: nc.sync.dma_start(
                            WOB[:, 2 * cc_:2 * cc_ + 2, :],
                            _ap3(ins["woT"], 256 * cc_, 2, P * D, D)))
                else:
                    nzh = 2 if h < 4 else (1 if h < 12 else 0)
                    for _ in range(nzh):
                        tb, oc = divmod(nz, 4)
                        zips.append(lambda tb_=tb, oc_=oc, n=nz: o_tile(
                            tb_, oc_, cc1, 0, n % 2))
                        nz += 1
                st = emit_scores(h, q0, qw, tiles, zips)
                if pend is not None:
                    emit_av(pend)
                pend = st
                if p_ == 1 and h == 11 and with_collective:
                    nc.gpsimd.collective_compute(
                        "ReduceScatter", ALU.add,
                        replica_groups=[[0, 1], [2, 3], [4, 5], [6, 7]],
                        ins=[cc1.opt()], outs=[cc1o.opt()])
            if p_ == 0:
                emit_av(pend)
                pend = None
                proj.__exit__(None, None, None)
                qz_cm.__exit__(None, None, None)
                po_cm = tc.tile_pool(name="pop", bufs=1, space="PSUM")
                pop = po_cm.__enter__()
        emit_av(pend)

        # ---------------- pass-2 O proj + RS2 ----------------------------
        for tb in range(4, 7):
            for oc in range(4):
                o_tile(tb, oc, cc2, 512, (tb * 4 + oc) % 2)
        if with_collective:
            nc.gpsimd.collective_compute(
                "ReduceScatter", ALU.add,
                replica_groups=[[0, 1], [2, 3], [4, 5], [6, 7]],
                ins=[cc2.opt()], outs=[cc2o.opt()])
            nc.gpsimd.dma_start(
                bass.AP(tensor=y.tensor, offset=y.offset,
                        ap=[[D, 256], [1, D]]),
                cc1o.opt())
            nc.gpsimd.dma_start(
                bass.AP(tensor=y.tensor, offset=y.offset + 256 * D,
                        ap=[[D, 96], [1, D]]),
                bass.AP(tensor=cc2o.tensor, offset=cc2o.offset,
                        ap=[[D, 96], [1, D]]))
            nc.scalar.dma_start(
                bass.AP(tensor=y.tensor, offset=y.offset + 352 * D,
                        ap=[[D, 96], [1, D]]),
                bass.AP(tensor=cc2o.tensor, offset=cc2o.offset + 96 * D,
                        ap=[[D, 96], [1, D]]))
            po_cm.__exit__(None, None, None)
        else:
            # debug path: no reduction; copy this core's partials
            nc.sync.dma_start(
                bass.AP(tensor=y.tensor, offset=y.offset,
                        ap=[[D, 256], [1, D]]),
                bass.AP(tensor=cc1.tensor, offset=cc1.offset,
                        ap=[[D, 256], [1, D]]))
            nc.sync.dma_start(
                bass.AP(tensor=y.tensor, offset=y.offset + 256 * D,
                        ap=[[D, 192], [1, D]]),
                bass.AP(tensor=cc2.tensor, offset=cc2.offset,
                        ap=[[D, 192], [1, D]]))
            po_cm.__exit__(None, None, None)
        att_cm.__exit__(None, None, None)


# ---------------------------------------------------------------- host side
_ROPE_PERM = np.concatenate([
    np.arange(0, 32, 2), np.arange(1, 32, 2),
    np.arange(32, 64, 2), np.arange(33, 64, 2)])


def make_in_maps(x, wq, wk, wv, wo, temporal_pos, structural_pos):
    import ml_dtypes
    bf16 = ml_dtypes.bfloat16
    x = np.asarray(x, dtype=np.float32)
    wq = np.asarray(wq, dtype=np.float32)
    wk = np.asarray(wk, dtype=np.float32)
    wv = np.asarray(wv, dtype=np.float32)
    wo = np.asarray(wo, dtype=np.float32)
    pt = np.asarray(temporal_pos).astype(np.float64)
    ps = np.asarray(structural_pos).astype(np.float64)
    inv = 1.0 / (10000.0 ** (np.arange(16) / 16.0))
    ct, st = np.cos(pt[:, None] * inv).T, np.sin(pt[:, None] * inv).T
    cs, ss = np.cos(ps[:, None] * inv).T, np.sin(ps[:, None] * inv).T
    Ct = np.concatenate([ct, ct, cs, cs] * 2).astype(bf16)     # [128, 896]
    St = np.concatenate([-st, st, -ss, ss] * 2).astype(bf16)

    wq_p = wq.reshape(HQ, HD, D)[:, _ROPE_PERM, :].reshape(D, D)
    wk_p = wk.reshape(HKV, HD, D)[:, _ROPE_PERM, :].reshape(HKV * HD, D)
    wqT = np.ascontiguousarray(wq_p.T).astype(bf16)   # [D, D]
    wkT = np.ascontiguousarray(wk_p.T).astype(bf16)   # [D, 512]
    wvT = np.ascontiguousarray(wv.T).astype(bf16)     # [D, 512]
    woT = np.ascontiguousarray(wo.T).astype(bf16)     # [D, D]

    in_maps = []
    for c in range(NCORES):
        b, g = divmod(c, 2)
        wkv = np.concatenate([wkT[:, g * KVO:(g + 1) * KVO],
                              wvT[:, g * KVO:(g + 1) * KVO]], axis=1)
        in_maps.append({
            "xT": np.ascontiguousarray(x[b].T).astype(bf16),
            "wqT": np.ascontiguousarray(wqT[:, g * GO:(g + 1) * GO]),
            "wkvT": np.ascontiguousarray(wkv),
            "woT": np.ascontiguousarray(woT[g * GO:(g + 1) * GO, :]),
            "Ct": Ct,
            "St": St,
        })
    return in_maps


def kernel(x, wq, wk, wv, wo, temporal_pos, structural_pos, _trace=False):
    nc = build_nc(with_collective=True)
    in_maps = make_in_maps(x, wq, wk, wv, wo, temporal_pos, structural_pos)
    res = bass_utils.run_bass_kernel_spmd(
        nc, in_maps, core_ids=list(range(NCORES)), trace=_trace)
    out = np.empty((B, L, D), np.float32)
    for b in range(B):
        e = np.asarray(res.results[2 * b]["y"]).astype(np.float32)
        o = np.asarray(res.results[2 * b + 1]["y"]).astype(np.float32)
        out[b, 0:256] = e[0:256]
        out[b, 256:512] = o[0:256]
        out[b, 512:704] = e[256:448]
        out[b, 704:896] = o[256:448]
    kernel.last_result = res
    return out


# revision 5
# speedup vs baseline: 1.0378x; 1.0160x over previous
"""GQA attention with 2D RoPE on 8 TRN2 NeuronCores — v3.

Sharding: batch data-parallel x4  X  head-group tensor-parallel x2.
Core c handles batch b=c//2 and head group g=c%2 (16 Q heads, 4 KV heads).
wo is row-sharded; the cross-pair reduction is done with two ROW-wise
ReduceScatters that write straight into the output tensor (no readback):
  RS1 over q rows [0:512)  -> even core y[0:256) = rows 0:256,
                              odd  core y[0:256) = rows 256:512
  RS2 over q rows [512:896) -> even y[256:448) = rows 512:704,
                               odd  y[256:448) = rows 704:896

v3 vs v2:
  - attention processed in two query passes (q 0:512 / 512:896) with
    exact-width causal score chunks (512-wide groups): 1008 matmuls
    total vs 1264.
  - causal masking via affine_select directly on the 128-wide diagonal
    blocks of U (no mask tiles).
  - O-projection for pass-1 rows zipped into pass-2 attention so RS1
    fires mid-pass-2; RS2 fires right after the last O tile; the
    collectives write y directly (zero tail after RS2).
  - Q projection (qblk 2-7) and WOB loads zipped into pass 1.
"""

import math
import numpy as np

import concourse.bass as bass
import concourse.tile as tile
import concourse.mybir as mybir
from concourse import bacc
from concourse import bass_utils

F32 = mybir.dt.float32
BF16 = mybir.dt.bfloat16
AF = mybir.ActivationFunctionType
ALU = mybir.AluOpType

B, L, D = 4, 896, 2048
HQ, HKV, HD = 32, 8, 64
NCORES = 8
GO = D // 2          # 1024 q-out dims per core
KVO = HKV * HD // 2  # 256 kv-out dims per core
NH = 16              # q heads per core
NKV = 4              # kv heads per core
P = 128
NI = D // P          # 16 contraction chunks
LB = L // P          # 7 key blocks
TOKC = ((0, 512), (512, 384))   # token chunks

# attention passes: (q0, qw, score tiles); tile = (kbs, slot_width)
PASSES = (
    (0, 512, (((0, 1), 512), ((2, 3), 256))),
    (512, 384, (((0, 1), 384), ((2, 3), 384), ((4, 5), 384), ((6,), 128))),
)

_NC_CACHE = {}
DEBUG_SKIP_AV = set()    # pass-2 kbs to drop from AV (debug only)
DEBUG_SKIP_SCORE = set()  # pass-2 kbs to skip scoring (debug only)


def build_nc(with_collective=True):
    key = (with_collective, tuple(sorted(DEBUG_SKIP_AV)),
           tuple(sorted(DEBUG_SKIP_SCORE)))
    if key in _NC_CACHE:
        return _NC_CACHE[key]
    nc = bacc.Bacc("TRN2", target_bir_lowering=False, debug=False,
                   num_devices=NCORES)
    ins = {
        "xT": nc.dram_tensor("xT", [D, L], BF16, kind="ExternalInput").ap(),
        "wqT": nc.dram_tensor("wqT", [D, GO], BF16, kind="ExternalInput").ap(),
        "wkvT": nc.dram_tensor("wkvT", [D, 2 * KVO], BF16,
                               kind="ExternalInput").ap(),
        "woT": nc.dram_tensor("woT", [GO, D], BF16, kind="ExternalInput").ap(),
        "Ct": nc.dram_tensor("Ct", [P, L], BF16, kind="ExternalInput").ap(),
        "St": nc.dram_tensor("St", [P, L], BF16, kind="ExternalInput").ap(),
    }
    y = nc.dram_tensor("y", [448, D], BF16, kind="ExternalOutput").ap()
    with tile.TileContext(nc) as tc:
        _build_kernel(nc, tc, ins, y, with_collective)
    nc.compile()
    _NC_CACHE[key] = nc
    return nc


def _ap3(dram_ap, row0, nrow_groups, group, ncols):
    """[128, nrow_groups, ncols] AP over dram rows row0.. in groups of 128."""
    return bass.AP(tensor=dram_ap.tensor,
                   offset=dram_ap.offset + row0 * ncols,
                   ap=[[ncols, P], [group, nrow_groups], [1, ncols]])


def _rope(nc, pool, t, C, S):
    """t = t*C + shuffle16(t)*S, fully in place."""
    shuf = pool.tile([P, L], BF16, tag="rope_shuf", name="rope_shuf")
    mask = [(p ^ 16) for p in range(32)]
    nc.vector.stream_shuffle(shuf[:], t[:], mask)
    nc.vector.tensor_mul(t[:], t[:], C[:])
    nc.vector.tensor_mul(shuf[:], shuf[:], S[:])
    nc.vector.tensor_add(t[:], t[:], shuf[:])


def _build_kernel(nc, tc, ins, y, with_collective):
    import contextlib
    ctx = contextlib.ExitStack()
    with ctx:
        const = ctx.enter_context(tc.tile_pool(name="const", bufs=1))
        big = ctx.enter_context(tc.tile_pool(name="big", bufs=1))
        WOB = big.tile([P, 8, D], BF16, tag="wob", name="wob")
        QT = [big.tile([P, L], BF16, tag=f"qt{i}", name=f"qt{i}")
              for i in range(8)]
        KTd = [big.tile([P, L], BF16, tag=f"kt{i}", name=f"kt{i}")
               for i in range(NKV)]
        Vext = [[big.tile([P, 192], BF16, tag=f"v{k}_{b_}",
                          name=f"v{k}_{b_}")
                  for b_ in range(LB)] for k in range(NKV)]
        AT = [big.tile([P, L], BF16, tag=f"at{i}", name=f"at{i}")
              for i in range(8)]

        ev = ctx.enter_context(tc.tile_pool(name="ev", bufs=2))
        upool = ctx.enter_context(tc.tile_pool(name="uatt", bufs=3))
        recpool = ctx.enter_context(tc.tile_pool(name="rec", bufs=2))
        otpool = ctx.enter_context(tc.tile_pool(name="ot", bufs=3))
        ccdram = ctx.enter_context(tc.tile_pool(name="ccdram", bufs=1,
                                                space="DRAM"))
        cc1 = ccdram.tile([512, D], BF16, tag="cc1", name="cc1")
        cc2 = ccdram.tile([384, D], BF16, tag="cc2", name="cc2")
        cc1o = ccdram.tile([256, D], BF16, tag="cc1o", name="cc1o")
        cc2o = ccdram.tile([192, D], BF16, tag="cc2o", name="cc2o")

        proj = tc.tile_pool(name="proj", bufs=1)
        prj = proj.__enter__()
        XT = prj.tile([P, NI, L], BF16, tag="xt", name="xt")
        WKV = prj.tile([P, NI, 2 * KVO], BF16, tag="wkv", name="wkv")
        WQ = prj.tile([P, NI, GO], BF16, tag="wq", name="wq")

        # ---- rope tables ----
        C = const.tile([P, L], BF16, tag="C", name="C")
        S = const.tile([P, L], BF16, tag="S", name="S")
        # ---- input DMAs (issue order = priority) ----
        nc.sync.dma_start(WKV[:, 0:1, :],
                          _ap3(ins["wkvT"], 0, 1, P * 2 * KVO, 2 * KVO))
        nc.sync.dma_start(XT[:, 0:1, :], _ap3(ins["xT"], 0, 1, P * L, L))
        nc.sync.dma_start(XT[:, 1:2, :], _ap3(ins["xT"], 128, 1, P * L, L))
        nc.sync.dma_start(C[:], ins["Ct"])
        nc.sync.dma_start(S[:], ins["St"])
        nc.sync.dma_start(WKV[:, 1:4, :],
                          _ap3(ins["wkvT"], 128, 3, P * 2 * KVO, 2 * KVO))
        for c8 in range(1, 8):
            nc.sync.dma_start(XT[:, 2 * c8:2 * c8 + 2, :],
                              _ap3(ins["xT"], 256 * c8, 2, P * L, L))
            if c8 < 4:
                nc.sync.dma_start(
                    WKV[:, 4 * c8:4 * c8 + 4, :],
                    _ap3(ins["wkvT"], 512 * c8, 4, P * 2 * KVO, 2 * KVO))
        for c2 in range(2):
            nc.sync.dma_start(WQ[:, 8 * c2:8 * c2 + 8, :],
                              _ap3(ins["wqT"], 1024 * c2, 8, P * GO, GO))
        # act-table prewarm + ones halves of Vext + causal triangle mask
        warm = const.tile([P, 1], BF16, tag="warm", name="warm")
        nc.scalar.activation(warm[:], warm[:], AF.Exp, scale=0.125)
        TRI = const.tile([P, P], BF16, tag="tri", name="tri")
        nc.gpsimd.memset(TRI[:], 1.0)
        nc.gpsimd.affine_select(out=TRI[:], in_=TRI[:],
                                compare_op=ALU.is_ge, fill=0.0, base=0,
                                channel_multiplier=-1, pattern=[[1, P]])
        for k in range(NKV):
            for b_ in range(LB):
                nc.gpsimd.memset(Vext[k][b_][:, 0:64], 1.0)
                nc.gpsimd.memset(Vext[k][b_][:, 128:192], 1.0)

        # ---------------- phase 1: V, K, Q(qblk 0-1) ---------------------
        with tc.tile_pool(name="ph1", bufs=1, space="PSUM") as ph1:
            psv = [ph1.tile([P, 512], F32, tag=f"p{j}", name=f"pv{j}")
                   for j in range(LB)]
            for i in range(NI):
                st, sp = (i == 0), (i == NI - 1)
                for b_ in range(LB):
                    nc.tensor.matmul(
                        psv[b_][:, 0:KVO], XT[:, i, b_ * P:(b_ + 1) * P],
                        WKV[:, i, KVO:2 * KVO], start=st, stop=sp)
            for b_ in (4, 5, 6, 0, 1, 2, 3):
                for k in range(NKV):
                    sl = psv[b_][:, k * 64:(k + 1) * 64]
                    if k % 2 == 0:
                        nc.vector.tensor_copy(Vext[k][b_][:, 64:128], sl)
                    else:
                        nc.scalar.copy(Vext[k][b_][:, 64:128], sl)

            _ktag = (7, 4, 5, 6)
            psk = [ph1.tile([P, 512], F32, tag=f"p{_ktag[ob * 2 + tci]}",
                            name=f"pk{ob}_{tci}")
                   for ob in range(2) for tci, (t0, tw) in enumerate(TOKC)]
            for i in range(NI):
                st, sp = (i == 0), (i == NI - 1)
                for ob in range(2):
                    for tci, (t0, tw) in enumerate(TOKC):
                        nc.tensor.matmul(
                            psk[ob * 2 + tci][:, 0:tw],
                            WKV[:, i, ob * P:(ob + 1) * P],
                            XT[:, i, t0:t0 + tw], start=st, stop=sp)
            for ob in range(2):
                roped = ev.tile([P, L], BF16, tag="roped", name=f"ktall{ob}")
                for tci, (t0, tw) in enumerate(TOKC):
                    nc.scalar.copy(roped[:, t0:t0 + tw],
                                   psk[ob * 2 + tci][:, 0:tw])
                _rope(nc, ev, roped, C, S)
                for sub in range(2):
                    k = ob * 2 + sub
                    src = roped[sub * 64:(sub + 1) * 64, :]
                    nc.sync.dma_start(KTd[k][0:64, :], src)
                    nc.sync.dma_start(KTd[k][64:128, :], src)

            psq01 = [ph1.tile([P, 512], F32, tag=f"p{qb * 2 + tci}",
                              name=f"pq{qb}_{tci}")
                     for qb in range(2) for tci, (t0, tw) in enumerate(TOKC)]
            for i in range(NI):
                st, sp = (i == 0), (i == NI - 1)
                for qb in range(2):
                    for tci, (t0, tw) in enumerate(TOKC):
                        nc.tensor.matmul(
                            psq01[qb * 2 + tci][:, 0:tw],
                            WQ[:, i, qb * P:(qb + 1) * P],
                            XT[:, i, t0:t0 + tw], start=st, stop=sp)
            for qb in range(2):
                for tci, (t0, tw) in enumerate(TOKC):
                    nc.scalar.copy(QT[qb][:, t0:t0 + tw],
                                   psq01[qb * 2 + tci][:, 0:tw])
                _rope(nc, ev, QT[qb], C, S)

        # ---------------- attention-phase psum pool ----------------------
        att_cm = tc.tile_pool(name="att", bufs=1, space="PSUM")
        ps = att_cm.__enter__()
        sctr = [0]

        def q_tile(qblk, tci):
            t0, tw = TOKC[tci]
            psq = qzp.tile([P, 512], F32, tag="qz", name=f"psq{qblk}_{tci}")
            for i in range(NI):
                nc.tensor.matmul(psq[:, 0:tw], WQ[:, i, qblk * P:(qblk + 1) * P],
                                 XT[:, i, t0:t0 + tw],
                                 start=(i == 0), stop=(i == NI - 1))
            nc.scalar.copy(QT[qblk][:, t0:t0 + tw], psq[:, 0:tw])
            if tci == 1:
                _rope(nc, ev, QT[qblk], C, S)

        def o_tile(tb, oc, cc, row0, cast):
            pso = pop.tile([P, 512], F32, tag=f"po{(tb * 4 + oc) % 2}",
                           name=f"pso{tb}_{oc}")
            for ic in range(8):
                nc.tensor.matmul(pso[:], AT[ic][:, tb * P:(tb + 1) * P],
                                 WOB[:, ic, oc * 512:(oc + 1) * 512],
                                 start=(ic == 0), stop=(ic == 7))
            ot = otpool.tile([P, 512], BF16, tag=f"ot{(tb * 4 + oc) % 3}",
                             name=f"ot{tb}_{oc}")
            if cast == 0:
                nc.vector.tensor_copy(ot[:], pso[:])
            else:
                nc.scalar.copy(ot[:], pso[:])
            dq = nc.sync if (cast == 1 or cc is cc1) else nc.scalar
            dq.dma_start(
                bass.AP(tensor=cc.tensor,
                        offset=cc.offset + (tb * P - row0) * D + oc * 512,
                        ap=[[D, P], [1, 512]]),
                ot[:])

        def emit_scores(h, q0, qw, tiles, zips=()):
            kv = h // 4
            qblk, qsub = divmod(h, 2)
            qoff = 64 * qsub
            zi = list(zips)
            Us = []
            for ti, (kbs, sw) in enumerate(tiles):
                ns = len(kbs)
                pss = ps.tile([P, ns, 512], F32, tag=f"sa{sctr[0] % 2}",
                              name=f"pss{h}_{q0}_{ti}")
                sctr[0] += 1
                for j, kb in enumerate(kbs):
                    w0 = max(q0, P * kb)
                    w = q0 + qw - w0
                    nc.tensor.matmul(
                        pss[:, j, 0:w],
                        KTd[kv][qoff:qoff + 64, kb * P:(kb + 1) * P],
                        QT[qblk][qoff:qoff + 64, w0:w0 + w],
                        start=True, stop=True, tile_position=(qoff, 0))
                if ti == 0 and zi:
                    zi.pop(0)()
                U = upool.tile([P, ns, sw], BF16, tag=f"u{ti % 2}",
                               name=f"u{h}_{q0}_{ti}")
                nc.scalar.activation(U[:, :, :], pss[:, :, 0:sw], AF.Exp,
                                     scale=0.125)
                for j, kb in enumerate(kbs):
                    if P * kb >= q0:   # diagonal block -> causal mask (DVE)
                        nc.vector.tensor_tensor(
                            U[:, j, 0:P], U[:, j, 0:P], TRI[:], op=ALU.mult)
                Us.append((kbs, U))
            for z in zi:
                z()
            return (h, q0, qw, tiles, Us)

        def emit_av(state):
            h, q0, qw, tiles, Us = state
            kv = h // 4
            qblk, qsub = divmod(h, 2)
            qoff, soff, vr = 64 * qsub, 64 - 64 * qsub, qsub
            nkb = sum(len(kbs) for kbs, _ in tiles)
            psav = ps.tile([P, 512], F32, tag=f"av{h % 2}",
                           name=f"psav{h}_{q0}")
            kbi = 0
            for kbs, U in Us:
                for j, kb in enumerate(kbs):
                    w0 = max(q0, P * kb)
                    w = q0 + qw - w0
                    off = w0 - q0
                    nc.tensor.matmul(
                        psav[:, off:off + w],
                        Vext[kv][kb][:, 64 * (1 - vr):64 * (1 - vr) + 128],
                        U[:, j, 0:w], start=(kbi == 0), stop=(kbi == nkb - 1),
                        skip_group_check=True)
                    kbi += 1
            recs = recpool.tile([P, 512], F32, tag="recs",
                                name=f"recs{h}_{q0}")
            nc.vector.reciprocal(recs[soff:soff + 64, 0:qw],
                                 psav[soff:soff + 64, 0:qw])
            rec = recpool.tile([P, 512], F32, tag="rec", name=f"rec{h}_{q0}")
            nc.vector.tensor_copy(rec[qoff:qoff + 64, 0:qw],
                                  recs[soff:soff + 64, 0:qw])
            nc.vector.tensor_tensor(
                AT[qblk][qoff:qoff + 64, q0:q0 + qw],
                psav[qoff:qoff + 64, 0:qw],
                rec[qoff:qoff + 64, 0:qw], op=ALU.mult)

        # ------- both passes, scores lead AV by one head -----------------
        qz_cm = tc.tile_pool(name="qzp", bufs=1, space="PSUM")
        qzp = qz_cm.__enter__()
        po_cm = None
        pend = None
        nz = 0
        for p_, (q0, qw, tiles) in enumerate(PASSES):
            for h in range(NH):
                zips = []
                if p_ == 0:
                    if 2 <= h < 14:
                        zips.append(lambda qb=2 + (h - 2) // 2,
                                    t=(h - 2) % 2: q_tile(qb, t))
                    if 4 <= h < 8:
                        c = h - 4
                        zips.append(lambda cc_=c: nc.sync.dma_start(
                            WOB[:, 2 * cc_:2 * cc_ + 2, :],
                            _ap3(ins["woT"], 256 * cc_, 2, P * D, D)))
                else:
                    nzh = 2 if h < 4 else (1 if h < 12 else 0)
                    for _ in range(nzh):
                        tb, oc = divmod(nz, 4)
                        zips.append(lambda tb_=tb, oc_=oc, n=nz: o_tile(
                            tb_, oc_, cc1, 0, n % 2))
                        nz += 1
                st = emit_scores(h, q0, qw, tiles, zips)
                if pend is not None:
                    emit_av(pend)
                pend = st
                if p_ == 1 and h == 11 and with_collective:
                    nc.gpsimd.collective_compute(
                        "ReduceScatter", ALU.add,
                        replica_groups=[[0, 1], [2, 3], [4, 5], [6, 7]],
                        ins=[cc1.opt()], outs=[cc1o.opt()])
            if p_ == 0:
                emit_av(pend)
                pend = None
                proj.__exit__(None, None, None)
                qz_cm.__exit__(None, None, None)
                po_cm = tc.tile_pool(name="pop", bufs=1, space="PSUM")
                pop = po_cm.__enter__()
        emit_av(pend)

        # ---------------- pass-2 O proj + RS2 ----------------------------
        for tb in range(4, 7):
            for oc in range(4):
                o_tile(tb, oc, cc2, 512, (tb * 4 + oc) % 2)
        if with_collective:
            nc.gpsimd.collective_compute(
                "ReduceScatter", ALU.add,
                replica_groups=[[0, 1], [2, 3], [4, 5], [6, 7]],
                ins=[cc2.opt()], outs=[cc2o.opt()])
            nc.gpsimd.dma_start(
                bass.AP(tensor=y.tensor, offset=y.offset,
                        ap=[[D, 256], [1, D]]),
                cc1o.opt())
            nc.gpsimd.dma_start(
                bass.AP(tensor=y.tensor, offset=y.offset + 256 * D,
                        ap=[[D, 96], [1, D]]),
                bass.AP(tensor=cc2o.tensor, offset=cc2o.offset,
                        ap=[[D, 96], [1, D]]))
            nc.scalar.dma_start(
                bass.AP(tensor=y.tensor, offset=y.offset + 352 * D,
                        ap=[[D, 96], [1, D]]),
                bass.AP(tensor=cc2o.tensor, offset=cc2o.offset + 96 * D,
                        ap=[[D, 96], [1, D]]))
            po_cm.__exit__(None, None, None)
        else:
            # debug path: no reduction; copy this core's partials
            nc.sync.dma_start(
                bass.AP(tensor=y.tensor, offset=y.offset,
                        ap=[[D, 256], [1, D]]),
                bass.AP(tensor=cc1.tensor, offset=cc1.offset,
                        ap=[[D, 256], [1, D]]))
            nc.sync.dma_start(
                bass.AP(tensor=y.tensor, offset=y.offset + 256 * D,
                        ap=[[D, 192], [1, D]]),
                bass.AP(tensor=cc2.tensor, offset=cc2.offset,
                        ap=[[D, 192], [1, D]]))
            po_cm.__exit__(None, None, None)
        att_cm.__exit__(None, None, None)


# ---------------------------------------------------------------- host side
_ROPE_PERM = np.concatenate([
    np.arange(0, 32, 2), np.arange(1, 32, 2),
    np.arange(32, 64, 2), np.arange(33, 64, 2)])


def make_in_maps(x, wq, wk, wv, wo, temporal_pos, structural_pos):
    import ml_dtypes
    bf16 = ml_dtypes.bfloat16
    x = np.asarray(x, dtype=np.float32)
    wq = np.asarray(wq, dtype=np.float32)
    wk = np.asarray(wk, dtype=np.float32)
    wv = np.asarray(wv, dtype=np.float32)
    wo = np.asarray(wo, dtype=np.float32)
    pt = np.asarray(temporal_pos).astype(np.float64)
    ps = np.asarray(structural_pos).astype(np.float64)
    inv = 1.0 / (10000.0 ** (np.arange(16) / 16.0))
    ct, st = np.cos(pt[:, None] * inv).T, np.sin(pt[:, None] * inv).T
    cs, ss = np.cos(ps[:, None] * inv).T, np.sin(ps[:, None] * inv).T
    Ct = np.concatenate([ct, ct, cs, cs] * 2).astype(bf16)     # [128, 896]
    St = np.concatenate([-st, st, -ss, ss] * 2).astype(bf16)

    wq_p = wq.reshape(HQ, HD, D)[:, _ROPE_PERM, :].reshape(D, D)
    wk_p = wk.reshape(HKV, HD, D)[:, _ROPE_PERM, :].reshape(HKV * HD, D)
    wqT = np.ascontiguousarray(wq_p.T).astype(bf16)   # [D, D]
    wkT = np.ascontiguousarray(wk_p.T).astype(bf16)   # [D, 512]
    wvT = np.ascontiguousarray(wv.T).astype(bf16)     # [D, 512]
    woT = np.ascontiguousarray(wo.T).astype(bf16)     # [D, D]

    in_maps = []
    for c in range(NCORES):
        b, g = divmod(c, 2)
        wkv = np.concatenate([wkT[:, g * KVO:(g + 1) * KVO],
                              wvT[:, g * KVO:(g + 1) * KVO]], axis=1)
        in_maps.append({
            "xT": np.ascontiguousarray(x[b].T).astype(bf16),
            "wqT": np.ascontiguousarray(wqT[:, g * GO:(g + 1) * GO]),
            "wkvT": np.ascontiguousarray(wkv),
            "woT": np.ascontiguousarray(woT[g * GO:(g + 1) * GO, :]),
            "Ct": Ct,
            "St": St,
        })
    return in_maps


def kernel(x, wq, wk, wv, wo, temporal_pos, structural_pos, _trace=False):
    nc = build_nc(with_collective=True)
    in_maps = make_in_maps(x, wq, wk, wv, wo, temporal_pos, structural_pos)
    res = bass_utils.run_bass_kernel_spmd(
        nc, in_maps, core_ids=list(range(NCORES)), trace=_trace)
    out = np.empty((B, L, D), np.float32)
    for b in range(B):
        e = np.asarray(res.results[2 * b]["y"]).astype(np.float32)
        o = np.asarray(res.results[2 * b + 1]["y"]).astype(np.float32)
        out[b, 0:256] = e[0:256]
        out[b, 256:512] = o[0:256]
        out[b, 512:704] = e[256:448]
        out[b, 704:896] = o[256:448]
    kernel.last_result = res
    return out


# revision 6
# speedup vs baseline: 1.0380x; 1.0002x over previous
"""GQA attention with 2D RoPE on 8 TRN2 NeuronCores — v3.

Sharding: batch data-parallel x4  X  head-group tensor-parallel x2.
Core c handles batch b=c//2 and head group g=c%2 (16 Q heads, 4 KV heads).
wo is row-sharded; the cross-pair reduction is done with two ROW-wise
ReduceScatters that write straight into the output tensor (no readback):
  RS1 over q rows [0:512)  -> even core y[0:256) = rows 0:256,
                              odd  core y[0:256) = rows 256:512
  RS2 over q rows [512:896) -> even y[256:448) = rows 512:704,
                               odd  y[256:448) = rows 704:896

v3 vs v2:
  - attention processed in two query passes (q 0:512 / 512:896) with
    exact-width causal score chunks (512-wide groups): 1008 matmuls
    total vs 1264.
  - causal masking via affine_select directly on the 128-wide diagonal
    blocks of U (no mask tiles).
  - O-projection for pass-1 rows zipped into pass-2 attention so RS1
    fires mid-pass-2; RS2 fires right after the last O tile; the
    collectives write y directly (zero tail after RS2).
  - Q projection (qblk 2-7) and WOB loads zipped into pass 1.
"""

import math
import numpy as np

import concourse.bass as bass
import concourse.tile as tile
import concourse.mybir as mybir
from concourse import bacc
from concourse import bass_utils

F32 = mybir.dt.float32
BF16 = mybir.dt.bfloat16
AF = mybir.ActivationFunctionType
ALU = mybir.AluOpType

B, L, D = 4, 896, 2048
HQ, HKV, HD = 32, 8, 64
NCORES = 8
GO = D // 2          # 1024 q-out dims per core
KVO = HKV * HD // 2  # 256 kv-out dims per core
NH = 16              # q heads per core
NKV = 4              # kv heads per core
P = 128
NI = D // P          # 16 contraction chunks
LB = L // P          # 7 key blocks
TOKC = ((0, 512), (512, 384))   # token chunks

# attention passes: (q0, qw, score tiles); tile = (kbs, slot_width)
PASSES = (
    (0, 512, (((0, 1), 512), ((2, 3), 256))),
    (512, 384, (((0, 1), 384), ((2, 3), 384), ((4, 5), 384), ((6,), 128))),
)

_NC_CACHE = {}
DEBUG_SKIP_AV = set()    # pass-2 kbs to drop from AV (debug only)
DEBUG_SKIP_SCORE = set()  # pass-2 kbs to skip scoring (debug only)


def build_nc(with_collective=True):
    key = (with_collective, tuple(sorted(DEBUG_SKIP_AV)),
           tuple(sorted(DEBUG_SKIP_SCORE)))
    if key in _NC_CACHE:
        return _NC_CACHE[key]
    nc = bacc.Bacc("TRN2", target_bir_lowering=False, debug=False,
                   num_devices=NCORES)
    ins = {
        "xT": nc.dram_tensor("xT", [D, L], BF16, kind="ExternalInput").ap(),
        "wqT": nc.dram_tensor("wqT", [D, GO], BF16, kind="ExternalInput").ap(),
        "wkvT": nc.dram_tensor("wkvT", [D, 2 * KVO], BF16,
                               kind="ExternalInput").ap(),
        "woT": nc.dram_tensor("woT", [GO, D], BF16, kind="ExternalInput").ap(),
        "Ct": nc.dram_tensor("Ct", [P, L], BF16, kind="ExternalInput").ap(),
        "St": nc.dram_tensor("St", [P, L], BF16, kind="ExternalInput").ap(),
    }
    y = nc.dram_tensor("y", [448, D], BF16, kind="ExternalOutput").ap()
    with tile.TileContext(nc) as tc:
        _build_kernel(nc, tc, ins, y, with_collective)
    nc.compile()
    _NC_CACHE[key] = nc
    return nc


def _ap3(dram_ap, row0, nrow_groups, group, ncols):
    """[128, nrow_groups, ncols] AP over dram rows row0.. in groups of 128."""
    return bass.AP(tensor=dram_ap.tensor,
                   offset=dram_ap.offset + row0 * ncols,
                   ap=[[ncols, P], [group, nrow_groups], [1, ncols]])


def _rope(nc, pool, t, C, S):
    """t = t*C + shuffle16(t)*S, fully in place."""
    shuf = pool.tile([P, L], BF16, tag="rope_shuf", name="rope_shuf")
    mask = [(p ^ 16) for p in range(32)]
    nc.vector.stream_shuffle(shuf[:], t[:], mask)
    nc.vector.tensor_mul(t[:], t[:], C[:])
    nc.vector.tensor_mul(shuf[:], shuf[:], S[:])
    nc.vector.tensor_add(t[:], t[:], shuf[:])


def _build_kernel(nc, tc, ins, y, with_collective):
    import contextlib
    ctx = contextlib.ExitStack()
    with ctx:
        const = ctx.enter_context(tc.tile_pool(name="const", bufs=1))
        big = ctx.enter_context(tc.tile_pool(name="big", bufs=1))
        WOB = big.tile([P, 8, D], BF16, tag="wob", name="wob")
        QT = [big.tile([P, L], BF16, tag=f"qt{i}", name=f"qt{i}")
              for i in range(8)]
        KTd = [big.tile([P, L], BF16, tag=f"kt{i}", name=f"kt{i}")
               for i in range(NKV)]
        Vext = [[big.tile([P, 192], BF16, tag=f"v{k}_{b_}",
                          name=f"v{k}_{b_}")
                  for b_ in range(LB)] for k in range(NKV)]
        AT = [big.tile([P, L], BF16, tag=f"at{i}", name=f"at{i}")
              for i in range(8)]

        ev = ctx.enter_context(tc.tile_pool(name="ev", bufs=2))
        upool = ctx.enter_context(tc.tile_pool(name="uatt", bufs=3))
        recpool = ctx.enter_context(tc.tile_pool(name="rec", bufs=2))
        otpool = ctx.enter_context(tc.tile_pool(name="ot", bufs=3))
        ccdram = ctx.enter_context(tc.tile_pool(name="ccdram", bufs=1,
                                                space="DRAM"))
        cc1 = ccdram.tile([512, D], BF16, tag="cc1", name="cc1")
        cc2 = ccdram.tile([384, D], BF16, tag="cc2", name="cc2")
        cc1o = ccdram.tile([256, D], BF16, tag="cc1o", name="cc1o")
        cc2o = ccdram.tile([192, D], BF16, tag="cc2o", name="cc2o")

        proj = tc.tile_pool(name="proj", bufs=1)
        prj = proj.__enter__()
        XT = prj.tile([P, NI, L], BF16, tag="xt", name="xt")
        WKV = prj.tile([P, NI, 2 * KVO], BF16, tag="wkv", name="wkv")
        WQ = prj.tile([P, NI, GO], BF16, tag="wq", name="wq")

        # ---- rope tables ----
        C = const.tile([P, L], BF16, tag="C", name="C")
        S = const.tile([P, L], BF16, tag="S", name="S")
        # ---- input DMAs (issue order = priority) ----
        nc.sync.dma_start(WKV[:, 0:1, :],
                          _ap3(ins["wkvT"], 0, 1, P * 2 * KVO, 2 * KVO))
        nc.sync.dma_start(XT[:, 0:1, :], _ap3(ins["xT"], 0, 1, P * L, L))
        nc.sync.dma_start(XT[:, 1:2, :], _ap3(ins["xT"], 128, 1, P * L, L))
        nc.sync.dma_start(C[:], ins["Ct"])
        nc.sync.dma_start(S[:], ins["St"])
        nc.sync.dma_start(WKV[:, 1:4, :],
                          _ap3(ins["wkvT"], 128, 3, P * 2 * KVO, 2 * KVO))
        for c8 in range(1, 8):
            nc.sync.dma_start(XT[:, 2 * c8:2 * c8 + 2, :],
                              _ap3(ins["xT"], 256 * c8, 2, P * L, L))
            if c8 < 4:
                nc.sync.dma_start(
                    WKV[:, 4 * c8:4 * c8 + 4, :],
                    _ap3(ins["wkvT"], 512 * c8, 4, P * 2 * KVO, 2 * KVO))
        for c2 in range(2):
            nc.sync.dma_start(WQ[:, 8 * c2:8 * c2 + 8, :],
                              _ap3(ins["wqT"], 1024 * c2, 8, P * GO, GO))
        # act-table prewarm + ones halves of Vext + causal triangle mask
        warm = const.tile([P, 1], BF16, tag="warm", name="warm")
        nc.scalar.activation(warm[:], warm[:], AF.Exp, scale=0.125)
        TRI = const.tile([P, P], BF16, tag="tri", name="tri")
        nc.gpsimd.memset(TRI[:], 1.0)
        nc.gpsimd.affine_select(out=TRI[:], in_=TRI[:],
                                compare_op=ALU.is_ge, fill=0.0, base=0,
                                channel_multiplier=-1, pattern=[[1, P]])
        for k in range(NKV):
            for b_ in range(LB):
                nc.gpsimd.memset(Vext[k][b_][:, 0:64], 1.0)
                nc.gpsimd.memset(Vext[k][b_][:, 128:192], 1.0)

        # ---------------- phase 1: V, K, Q(qblk 0-1) ---------------------
        with tc.tile_pool(name="ph1", bufs=1, space="PSUM") as ph1:
            psv = [ph1.tile([P, 512], F32, tag=f"p{j}", name=f"pv{j}")
                   for j in range(LB)]
            for i in range(NI):
                st, sp = (i == 0), (i == NI - 1)
                for b_ in range(LB):
                    nc.tensor.matmul(
                        psv[b_][:, 0:KVO], XT[:, i, b_ * P:(b_ + 1) * P],
                        WKV[:, i, KVO:2 * KVO], start=st, stop=sp)
            for b_ in (4, 5, 6, 0, 1, 2, 3):
                for k in range(NKV):
                    sl = psv[b_][:, k * 64:(k + 1) * 64]
                    if k % 2 == 0:
                        nc.vector.tensor_copy(Vext[k][b_][:, 64:128], sl)
                    else:
                        nc.scalar.copy(Vext[k][b_][:, 64:128], sl)

            _ktag = (7, 4, 5, 6)
            psk = [ph1.tile([P, 512], F32, tag=f"p{_ktag[ob * 2 + tci]}",
                            name=f"pk{ob}_{tci}")
                   for ob in range(2) for tci, (t0, tw) in enumerate(TOKC)]
            for i in range(NI):
                st, sp = (i == 0), (i == NI - 1)
                for ob in range(2):
                    for tci, (t0, tw) in enumerate(TOKC):
                        nc.tensor.matmul(
                            psk[ob * 2 + tci][:, 0:tw],
                            WKV[:, i, ob * P:(ob + 1) * P],
                            XT[:, i, t0:t0 + tw], start=st, stop=sp)
            for ob in range(2):
                roped = ev.tile([P, L], BF16, tag="roped", name=f"ktall{ob}")
                for tci, (t0, tw) in enumerate(TOKC):
                    nc.scalar.copy(roped[:, t0:t0 + tw],
                                   psk[ob * 2 + tci][:, 0:tw])
                _rope(nc, ev, roped, C, S)
                for sub in range(2):
                    k = ob * 2 + sub
                    src = roped[sub * 64:(sub + 1) * 64, :]
                    nc.sync.dma_start(KTd[k][0:64, :], src)
                    nc.sync.dma_start(KTd[k][64:128, :], src)

            psq01 = [ph1.tile([P, 512], F32, tag=f"p{qb * 2 + tci}",
                              name=f"pq{qb}_{tci}")
                     for qb in range(2) for tci, (t0, tw) in enumerate(TOKC)]
            for i in range(NI):
                st, sp = (i == 0), (i == NI - 1)
                for qb in range(2):
                    for tci, (t0, tw) in enumerate(TOKC):
                        nc.tensor.matmul(
                            psq01[qb * 2 + tci][:, 0:tw],
                            WQ[:, i, qb * P:(qb + 1) * P],
                            XT[:, i, t0:t0 + tw], start=st, stop=sp)
            for qb in range(2):
                for tci, (t0, tw) in enumerate(TOKC):
                    nc.scalar.copy(QT[qb][:, t0:t0 + tw],
                                   psq01[qb * 2 + tci][:, 0:tw])
                _rope(nc, ev, QT[qb], C, S)

        # ---------------- attention-phase psum pool ----------------------
        att_cm = tc.tile_pool(name="att", bufs=1, space="PSUM")
        ps = att_cm.__enter__()
        sctr = [0]

        def q_tile(qblk, tci):
            t0, tw = TOKC[tci]
            psq = qzp.tile([P, 512], F32, tag="qz", name=f"psq{qblk}_{tci}")
            for i in range(NI):
                nc.tensor.matmul(psq[:, 0:tw], WQ[:, i, qblk * P:(qblk + 1) * P],
                                 XT[:, i, t0:t0 + tw],
                                 start=(i == 0), stop=(i == NI - 1))
            nc.scalar.copy(QT[qblk][:, t0:t0 + tw], psq[:, 0:tw])
            if tci == 1:
                _rope(nc, ev, QT[qblk], C, S)

        def o_tile(tb, oc, cc, row0, cast):
            pso = pop.tile([P, 512], F32, tag=f"po{(tb * 4 + oc) % 2}",
                           name=f"pso{tb}_{oc}")
            for ic in range(8):
                nc.tensor.matmul(pso[:], AT[ic][:, tb * P:(tb + 1) * P],
                                 WOB[:, ic, oc * 512:(oc + 1) * 512],
                                 start=(ic == 0), stop=(ic == 7))
            ot = otpool.tile([P, 512], BF16, tag=f"ot{(tb * 4 + oc) % 3}",
                             name=f"ot{tb}_{oc}")
            if cast == 0:
                nc.vector.tensor_copy(ot[:], pso[:])
            else:
                nc.scalar.copy(ot[:], pso[:])
            dq = nc.sync if (cast == 1 or cc is cc1) else nc.scalar
            dq.dma_start(
                bass.AP(tensor=cc.tensor,
                        offset=cc.offset + (tb * P - row0) * D + oc * 512,
                        ap=[[D, P], [1, 512]]),
                ot[:])

        def emit_scores(h, q0, qw, tiles, zips=()):
            kv = h // 4
            qblk, qsub = divmod(h, 2)
            qoff = 64 * qsub
            zi = list(zips)
            Us = []
            for ti, (kbs, sw) in enumerate(tiles):
                ns = len(kbs)
                pss = ps.tile([P, ns, 512], F32, tag=f"sa{sctr[0] % 2}",
                              name=f"pss{h}_{q0}_{ti}")
                sctr[0] += 1
                for j, kb in enumerate(kbs):
                    w0 = max(q0, P * kb)
                    w = q0 + qw - w0
                    nc.tensor.matmul(
                        pss[:, j, 0:w],
                        KTd[kv][qoff:qoff + 64, kb * P:(kb + 1) * P],
                        QT[qblk][qoff:qoff + 64, w0:w0 + w],
                        start=True, stop=True, tile_position=(qoff, 0))
                if ti == 0 and zi:
                    zi.pop(0)()
                U = upool.tile([P, ns, sw], BF16, tag=f"u{ti % 2}",
                               name=f"u{h}_{q0}_{ti}")
                nc.scalar.activation(U[:, :, :], pss[:, :, 0:sw], AF.Exp,
                                     scale=0.125)
                for j, kb in enumerate(kbs):
                    if P * kb >= q0:   # diagonal block -> causal mask (DVE)
                        nc.vector.tensor_tensor(
                            U[:, j, 0:P], U[:, j, 0:P], TRI[:], op=ALU.mult)
                Us.append((kbs, U))
            for z in zi:
                z()
            return (h, q0, qw, tiles, Us)

        def emit_av(state):
            h, q0, qw, tiles, Us = state
            kv = h // 4
            qblk, qsub = divmod(h, 2)
            qoff, soff, vr = 64 * qsub, 64 - 64 * qsub, qsub
            nkb = sum(len(kbs) for kbs, _ in tiles)
            psav = ps.tile([P, 512], F32, tag=f"av{h % 2}",
                           name=f"psav{h}_{q0}")
            kbi = 0
            for kbs, U in Us:
                for j, kb in enumerate(kbs):
                    w0 = max(q0, P * kb)
                    w = q0 + qw - w0
                    off = w0 - q0
                    nc.tensor.matmul(
                        psav[:, off:off + w],
                        Vext[kv][kb][:, 64 * (1 - vr):64 * (1 - vr) + 128],
                        U[:, j, 0:w], start=(kbi == 0), stop=(kbi == nkb - 1),
                        skip_group_check=True)
                    kbi += 1
            recs = recpool.tile([P, 512], F32, tag="recs",
                                name=f"recs{h}_{q0}")
            nc.vector.reciprocal(recs[soff:soff + 64, 0:qw],
                                 psav[soff:soff + 64, 0:qw])
            rec = recpool.tile([P, 512], F32, tag="rec", name=f"rec{h}_{q0}")
            nc.vector.tensor_copy(rec[qoff:qoff + 64, 0:qw],
                                  recs[soff:soff + 64, 0:qw])
            nc.vector.tensor_tensor(
                AT[qblk][qoff:qoff + 64, q0:q0 + qw],
                psav[qoff:qoff + 64, 0:qw],
                rec[qoff:qoff + 64, 0:qw], op=ALU.mult)

        # ------- both passes, scores lead AV by one head -----------------
        qz_cm = tc.tile_pool(name="qzp", bufs=1, space="PSUM")
        qzp = qz_cm.__enter__()
        po_cm = None
        pend = None
        nz = 0
        for p_, (q0, qw, tiles) in enumerate(PASSES):
            for h in range(NH):
                zips = []
                if p_ == 0:
                    if 2 <= h < 14:
                        zips.append(lambda qb=2 + (h - 2) // 2,
                                    t=(h - 2) % 2: q_tile(qb, t))
                    if 4 <= h < 8:
                        c = h - 4
                        zips.append(lambda cc_=c: nc.sync.dma_start(
                            WOB[:, 2 * cc_:2 * cc_ + 2, :],
                            _ap3(ins["woT"], 256 * cc_, 2, P * D, D)))
                else:
                    nzh = 2 if h < 6 else (1 if h < 10 else 0)
                    for _ in range(nzh):
                        tb, oc = divmod(nz, 4)
                        zips.append(lambda tb_=tb, oc_=oc, n=nz: o_tile(
                            tb_, oc_, cc1, 0, n % 2))
                        nz += 1
                st = emit_scores(h, q0, qw, tiles, zips)
                if pend is not None:
                    emit_av(pend)
                pend = st
                if p_ == 1 and h == 11 and with_collective:
                    nc.gpsimd.collective_compute(
                        "ReduceScatter", ALU.add,
                        replica_groups=[[0, 1], [2, 3], [4, 5], [6, 7]],
                        ins=[cc1.opt()], outs=[cc1o.opt()])
            if p_ == 0:
                emit_av(pend)
                pend = None
                proj.__exit__(None, None, None)
                qz_cm.__exit__(None, None, None)
                po_cm = tc.tile_pool(name="pop", bufs=1, space="PSUM")
                pop = po_cm.__enter__()
        emit_av(pend)

        # ---------------- pass-2 O proj + RS2 ----------------------------
        for tb in range(4, 7):
            for oc in range(4):
                o_tile(tb, oc, cc2, 512, (tb * 4 + oc) % 2)
        if with_collective:
            nc.gpsimd.collective_compute(
                "ReduceScatter", ALU.add,
                replica_groups=[[0, 1], [2, 3], [4, 5], [6, 7]],
                ins=[cc2.opt()], outs=[cc2o.opt()])
            nc.gpsimd.dma_start(
                bass.AP(tensor=y.tensor, offset=y.offset,
                        ap=[[D, 256], [1, D]]),
                cc1o.opt())
            nc.gpsimd.dma_start(
                bass.AP(tensor=y.tensor, offset=y.offset + 256 * D,
                        ap=[[D, 96], [1, D]]),
                bass.AP(tensor=cc2o.tensor, offset=cc2o.offset,
                        ap=[[D, 96], [1, D]]))
            nc.scalar.dma_start(
                bass.AP(tensor=y.tensor, offset=y.offset + 352 * D,
                        ap=[[D, 96], [1, D]]),
                bass.AP(tensor=cc2o.tensor, offset=cc2o.offset + 96 * D,
                        ap=[[D, 96], [1, D]]))
            po_cm.__exit__(None, None, None)
        else:
            # debug path: no reduction; copy this core's partials
            nc.sync.dma_start(
                bass.AP(tensor=y.tensor, offset=y.offset,
                        ap=[[D, 256], [1, D]]),
                bass.AP(tensor=cc1.tensor, offset=cc1.offset,
                        ap=[[D, 256], [1, D]]))
            nc.sync.dma_start(
                bass.AP(tensor=y.tensor, offset=y.offset + 256 * D,
                        ap=[[D, 192], [1, D]]),
                bass.AP(tensor=cc2.tensor, offset=cc2.offset,
                        ap=[[D, 192], [1, D]]))
            po_cm.__exit__(None, None, None)
        att_cm.__exit__(None, None, None)


# ---------------------------------------------------------------- host side
_ROPE_PERM = np.concatenate([
    np.arange(0, 32, 2), np.arange(1, 32, 2),
    np.arange(32, 64, 2), np.arange(33, 64, 2)])


def make_in_maps(x, wq, wk, wv, wo, temporal_pos, structural_pos):
    import ml_dtypes
    bf16 = ml_dtypes.bfloat16
    x = np.asarray(x, dtype=np.float32)
    wq = np.asarray(wq, dtype=np.float32)
    wk = np.asarray(wk, dtype=np.float32)
    wv = np.asarray(wv, dtype=np.float32)
    wo = np.asarray(wo, dtype=np.float32)
    pt = np.asarray(temporal_pos).astype(np.float64)
    ps = np.asarray(structural_pos).astype(np.float64)
    inv = 1.0 / (10000.0 ** (np.arange(16) / 16.0))
    ct, st = np.cos(pt[:, None] * inv).T, np.sin(pt[:, None] * inv).T
    cs, ss = np.cos(ps[:, None] * inv).T, np.sin(ps[:, None] * inv).T
    Ct = np.concatenate([ct, ct, cs, cs] * 2).astype(bf16)     # [128, 896]
    St = np.concatenate([-st, st, -ss, ss] * 2).astype(bf16)

    wq_p = wq.reshape(HQ, HD, D)[:, _ROPE_PERM, :].reshape(D, D)
    wk_p = wk.reshape(HKV, HD, D)[:, _ROPE_PERM, :].reshape(HKV * HD, D)
    wqT = np.ascontiguousarray(wq_p.T).astype(bf16)   # [D, D]
    wkT = np.ascontiguousarray(wk_p.T).astype(bf16)   # [D, 512]
    wvT = np.ascontiguousarray(wv.T).astype(bf16)     # [D, 512]
    woT = np.ascontiguousarray(wo.T).astype(bf16)     # [D, D]

    in_maps = []
    for c in range(NCORES):
        b, g = divmod(c, 2)
        wkv = np.concatenate([wkT[:, g * KVO:(g + 1) * KVO],
                              wvT[:, g * KVO:(g + 1) * KVO]], axis=1)
        in_maps.append({
            "xT": np.ascontiguousarray(x[b].T).astype(bf16),
            "wqT": np.ascontiguousarray(wqT[:, g * GO:(g + 1) * GO]),
            "wkvT": np.ascontiguousarray(wkv),
            "woT": np.ascontiguousarray(woT[g * GO:(g + 1) * GO, :]),
            "Ct": Ct,
            "St": St,
        })
    return in_maps


def kernel(x, wq, wk, wv, wo, temporal_pos, structural_pos, _trace=False):
    nc = build_nc(with_collective=True)
    in_maps = make_in_maps(x, wq, wk, wv, wo, temporal_pos, structural_pos)
    res = bass_utils.run_bass_kernel_spmd(
        nc, in_maps, core_ids=list(range(NCORES)), trace=_trace)
    out = np.empty((B, L, D), np.float32)
    for b in range(B):
        e = np.asarray(res.results[2 * b]["y"]).astype(np.float32)
        o = np.asarray(res.results[2 * b + 1]["y"]).astype(np.float32)
        out[b, 0:256] = e[0:256]
        out[b, 256:512] = o[0:256]
        out[b, 512:704] = e[256:448]
        out[b, 704:896] = o[256:448]
    kernel.last_result = res
    return out
